# revision 1
# baseline (speedup 1.0000x reference)
"""Multi-head causal self-attention with RoPE on 8 Trainium2 NeuronCores.

Problem: x:(4,2048,1024) f32, 16 heads, d_k=64, causal, RoPE theta=1e4,
out = softmax(rope(q) rope(k)^T / 8, causal) v, then out-proj.

Sharding: core c handles batch c//2 and heads 8*(c%2) .. 8*(c%2)+8.
Each core computes QKV for its 8 heads (row-sliced weights), causal
attention, and a partial out-projection y_part = attnout_slice @ WoT_slice.
Host sums the two partials per batch.

Device layouts (per core):
  xT   [i, s]     - transposed activations (PE-transposed on device)
  qT,kT[hd, s]    - projections in transposed layout (RoPE'd in place)
  v    [s, hd]+1s - natural layout with a ones column (softmax denominator)
  scoresT[sk, sq] - so exp tiles feed attn@V directly as moving operand
  attnoutT[hd, s] - feeds out-proj; output written as yT[o, s]

The per-head d_k dims of Wq/Wk are host-permuted (evens then odds) so RoPE
becomes the rotate-half form; scores are invariant to this permutation.
"""

from contextlib import ExitStack

import numpy as np

import concourse.tile as tile
from concourse import bacc, mybir
from concourse.bass_utils import run_bass_kernel_spmd

F32 = mybir.dt.float32
F32R = mybir.dt.float32r
AF = mybir.ActivationFunctionType

D_MODEL = 1024
SEQ = 2048
BATCH = 4
N_HEADS = 16
DK = 64
N_CORES = 8
HPC = 8            # heads per core
HD = HPC * DK      # 512 head-dims per core
P = 128
SC = 512           # seq chunk (matmul moving dim)
NSC = SEQ // SC    # 4
NST = SEQ // P     # 16
NIC = D_MODEL // P # 8
NH4 = HD // P      # 4  (128-dim tiles = 2 heads each)


ABLATE = set()  # dev-only: phase names to skip ("attn", "p5", "rope", "mask")


def _r(ap):
    return ap.bitcast(F32R)


def build_nc():
    nc = bacc.Bacc("TRN2", target_bir_lowering=False, debug=False)

    x_d = nc.dram_tensor("x", [SEQ, D_MODEL], F32, kind="ExternalInput").ap()
    wq_d = nc.dram_tensor("wqT", [D_MODEL, HD], F32, kind="ExternalInput").ap()
    wk_d = nc.dram_tensor("wkT", [D_MODEL, HD], F32, kind="ExternalInput").ap()
    wv_d = nc.dram_tensor("wvT", [D_MODEL, HD], F32, kind="ExternalInput").ap()
    wo_d = nc.dram_tensor("woT", [HD, D_MODEL], F32, kind="ExternalInput").ap()
    cos_d = nc.dram_tensor("cosw", [P, SEQ], F32, kind="ExternalInput").ap()
    sin_d = nc.dram_tensor("sinw", [P, SEQ], F32, kind="ExternalInput").ap()
    mask_d = nc.dram_tensor("mask", [P, P], F32, kind="ExternalInput").ap()
    id_d = nc.dram_tensor("ident", [P, P], F32, kind="ExternalInput").ap()
    ones_d = nc.dram_tensor("ones", [P, NST * HPC], F32, kind="ExternalInput").ap()
    y_d = nc.dram_tensor("yT", [D_MODEL, SEQ], F32, kind="ExternalOutput").ap()

    with tile.TileContext(nc) as tc:
        with ExitStack() as ctx:
            _emit(ctx, tc, x_d, wq_d, wk_d, wv_d, wo_d, cos_d, sin_d, mask_d,
                  id_d, ones_d, y_d)
    nc.compile()
    return nc


def _emit(ctx, tc, x_d, wq_d, wk_d, wv_d, wo_d, cos_d, sin_d, mask_d, id_d,
          ones_d, y_d):
    nc = tc.nc

    # DRAM staging for attention output (saves SBUF for phase 5)
    attno_d = nc.dram_tensor("attno_stage", [NH4, P, SEQ], F32).ap()

    persist = ctx.enter_context(tc.tile_pool(name="persist", bufs=1))

    # RoPE tables / causal masks (persist; ~24.5 KiB/part). Loaded on the
    # scalar queue so the x rows (sync queue) arrive first at kernel start.
    cos_sb = persist.tile([P, SEQ], F32, tag="cos")
    sin_sb = persist.tile([P, SEQ], F32, tag="sin")
    mask_sb = persist.tile([P, P], F32, tag="mask")
    v_sb = persist.tile([P, NST, HPC, 66], F32R, tag="v")

    with tc.tile_pool(name="xTp", bufs=1) as xT_pool:
        # ------------- Phase 0: x -> xT, and v projection (all heads) ----
        xT = xT_pool.tile([P, NIC, SEQ], F32R, tag="xT")   # 64 KiB/part
        with tc.tile_pool(name="xrow", bufs=6) as xrow_pool, \
             tc.tile_pool(name="wvp", bufs=1) as wv_pool, \
             tc.tile_pool(name="pst", bufs=4, space="PSUM") as pst_pool, \
             tc.tile_pool(name="psv", bufs=3, space="PSUM") as psv_pool:
            ident = wv_pool.tile([P, P], F32, tag="ident")
            nc.sync.dma_start(ident, id_d)
            wv_sb = wv_pool.tile([P, NIC, HD], F32R, tag="wv")
            nc.scalar.dma_start(wv_sb,
                                wv_d.rearrange("(ic p) o -> p ic o", p=P).bitcast(F32R))
            nc.scalar.dma_start(
                v_sb[:, :, :, 64:65],
                ones_d.rearrange("p (a b) -> p a b", a=NST).bitcast(F32R))
            nc.scalar.dma_start(cos_sb, cos_d)
            nc.scalar.dma_start(sin_sb, sin_d)
            nc.scalar.dma_start(mask_sb, mask_d)
            for st in range(NST):
                xrow = xrow_pool.tile([P, D_MODEL], F32, tag="xrow")
                nc.sync.dma_start(xrow, x_d[P * st:P * (st + 1), :])
                for ic in range(NIC):
                    pst = pst_pool.tile([P, P], F32, tag="pst")
                    nc.tensor.transpose(pst, xrow[:, P * ic:P * (ic + 1)],
                                        ident)
                    dst = xT[:, ic, P * st:P * (st + 1)]
                    if ic % 2:
                        nc.scalar.activation(dst, pst, func=AF.Copy)
                    else:
                        nc.vector.tensor_copy(dst, pst)
            for st in range(NST):
                psv = psv_pool.tile([P, HD // 2], F32, tag="psv",
                                    name=f"psv_{st}")
                for ic in range(NIC):
                    nc.tensor.matmul(
                        psv, lhsT=_r(xT[:, ic, P * st:P * (st + 1)]),
                        rhs=_r(wv_sb[:, ic, 0:HD // 2]),
                        start=(ic == 0), stop=(ic == NIC - 1))
                nc.scalar.activation(
                    v_sb[:, st, 0:HPC // 2, 0:64],
                    psv[:].rearrange("p (h d) -> p h d", h=HPC // 2),
                    func=AF.Copy)


        # ------------- Phases 1..4: per 2-head group: proj + rope + attn -
        with tc.tile_pool(name="wqk", bufs=3) as wqk_pool, \
             tc.tile_pool(name="qk", bufs=2) as qk_pool, \
             tc.tile_pool(name="rope", bufs=1) as rope_pool, \
             tc.tile_pool(name="exp", bufs=4) as exp_pool, \
             tc.tile_pool(name="nrm", bufs=3) as nrm_pool, \
             tc.tile_pool(name="ps2", bufs=3, space="PSUM") as ps2_pool, \
             tc.tile_pool(name="psatt", bufs=2, space="PSUM") as psatt_pool:

            def emit_proj(h4):
                qkT = {}
                for name, w_d in (("q", wq_d), ("k", wk_d)):
                    w_t = wqk_pool.tile([P, NIC, P], F32R, tag="wqk")
                    nc.sync.dma_start(
                        w_t, w_d.rearrange("(ic p) o -> p ic o",
                                           p=P)[:, :, P * h4:P * (h4 + 1)].bitcast(F32R))
                    dstT = qk_pool.tile([P, SEQ], F32R, tag=f"{name}T",
                                        name=f"{name}T_{h4}")
                    qkT[name] = dstT
                    # RoPE fused with psum evacuation:
                    #   dstT = (ps2 * cos);  swp = partition-swapped raw ps2
                    #   swp *= sin' (gpsimd);  dstT += swp
                    swp = rope_pool.tile([P, SEQ], F32, tag="swp",
                                         name=f"swp_{h4}_{name}")
                    rope_on = "rope" not in ABLATE
                    for scp in range(2):   # pairs of s-chunks, 1024 wide
                        ps2 = ps2_pool.tile([P, 2 * SC], F32, tag="ps2",
                                            name=f"ps2p_{h4}_{name}_{scp}")
                        for half in range(2):
                            sc = 2 * scp + half
                            for ic in range(NIC):
                                nc.tensor.matmul(
                                    ps2[:, SC * half:SC * (half + 1)],
                                    lhsT=_r(w_t[:, ic, :]),
                                    rhs=_r(xT[:, ic, SC * sc:SC * (sc + 1)]),
                                    start=(ic == 0), stop=(ic == NIC - 1))
                        chunk = slice(2 * SC * scp, 2 * SC * (scp + 1))
                        nc.vector.tensor_copy(dstT[:, chunk], ps2)
                        if rope_on:
                            for (o, i) in ((0, 32), (32, 0), (64, 96),
                                           (96, 64)):
                                nc.sync.dma_start(
                                    swp[o:o + 32, chunk],
                                    dstT[i:i + 32, chunk].bitcast(F32))
                    if rope_on:
                        nc.vector.tensor_mul(dstT, dstT, cos_sb)
                        nc.gpsimd.tensor_mul(swp, swp, sin_sb)
                        nc.vector.tensor_add(dstT, dstT, swp)
                return qkT

            def emit_attn(h4, qkT):
                # ---- attention for the two heads in this group ----
                # j (sq chunk) outer; sk-tiles t paired two per 2-bank psum:
                # scoresT for (t, t+1) side by side -> one exp -> two attnV
                # accumulations into psatt[j].  Diagonal handling:
                #   pair (4j, 4j+1): full exp; zero cols [512,640); band
                #     masks at [0:128] (r=0) and [640:768] (r=1)
                #   pair (4j+2, 4j+3): halves restricted to >= 256; two exps;
                #     zero [768,896); bands at [256:384] and [896:1024]
                masked = "mask" not in ABLATE
                for j in range(NSC):
                    for hp in range(2 * ("attn" not in ABLATE)):
                        h = 2 * h4 + hp
                        qh = qkT["q"][64 * hp:64 * hp + 64, :]
                        kh = qkT["k"][64 * hp:64 * hp + 64, :]
                        psj = psatt_pool.tile([65, SC], F32, tag="psatt",
                                              name=f"psatt_{h}_{j}")
                        tmax = 4 * j + 3
                        for tp in range(2 * j + 2):
                            t0 = 2 * tp
                            diag = t0 - 4 * j   # -4j..0..2: >=0 on diagonal
                            kind = ("full" if diag < 0 else
                                    "d01" if diag == 0 else "d23")
                            n0 = 2 * P if (kind == "d23" and masked) else 0
                            ps2 = ps2_pool.tile(
                                [P, 2 * SC], F32, tag="ps2",
                                name=f"ps2a_{h}_{j}_{tp}")
                            for m in range(2):
                                t = t0 + m
                                nc.tensor.matmul(
                                    ps2[:, SC * m + n0:SC * (m + 1)],
                                    lhsT=_r(kh[:, P * t:P * (t + 1)]),
                                    rhs=_r(qh[:, SC * j + n0:SC * (j + 1)]),
                                    start=True, stop=True)
                            exp2 = exp_pool.tile([P, 2 * SC], F32R,
                                                 tag="exp",
                                                 name=f"exp_{h}_{j}_{tp}")
                            if kind == "d23":
                                # one ACT op over both 256-wide valid halves
                                nc.scalar.activation(
                                    exp2[:].rearrange(
                                        "p (b c) -> p b c", b=2)[:, :, n0:SC],
                                    ps2[:].rearrange(
                                        "p (b c) -> p b c", b=2)[:, :, n0:SC],
                                    func=AF.Exp, scale=0.125)
                            else:
                                nc.scalar.activation(
                                    exp2, ps2, func=AF.Exp, scale=0.125)
                            if masked and kind == "d01":
                                nc.gpsimd.tensor_scalar_mul(
                                    exp2[:, SC:SC + P], exp2[:, SC:SC + P],
                                    0.0)
                                nc.gpsimd.tensor_mul(
                                    exp2[:, 0:P], exp2[:, 0:P], mask_sb)
                                nc.gpsimd.tensor_mul(
                                    exp2[:, SC + P:SC + 2 * P],
                                    exp2[:, SC + P:SC + 2 * P], mask_sb)
                            elif masked and kind == "d23":
                                nc.gpsimd.tensor_scalar_mul(
                                    exp2[:, SC + n0:SC + 3 * P],
                                    exp2[:, SC + n0:SC + 3 * P], 0.0)
                                nc.gpsimd.tensor_mul(
                                    exp2[:, n0:n0 + P], exp2[:, n0:n0 + P],
                                    mask_sb)
                                nc.gpsimd.tensor_mul(
                                    exp2[:, SC + 3 * P:2 * SC],
                                    exp2[:, SC + 3 * P:2 * SC], mask_sb)
                            for m in range(2):
                                t = t0 + m
                                out_ap = psj[:] if n0 == 0 \
                                    else psj[:, n0:]
                                nc.tensor.matmul(
                                    out_ap, lhsT=_r(v_sb[:, t, h, 0:65]),
                                    rhs=_r(exp2[:, SC * m + n0:SC * (m + 1)]),
                                    start=(t == 0), stop=(t == tmax))
                        # normalize + store this sq chunk
                        rec = nrm_pool.tile([1, SC], F32, tag="rec",
                                            name=f"rec_{h}_{j}")
                        nc.vector.reciprocal(rec, psj[64:65, :])
                        rbc = nrm_pool.tile([64, SC], F32, tag="rbc",
                                            name=f"rbc_{h}_{j}")
                        nc.gpsimd.partition_broadcast(rbc, rec[0:1, :])
                        ao_t = nrm_pool.tile([64, SC], F32, tag="ao",
                                             name=f"ao_{h}_{j}")
                        nc.vector.tensor_mul(ao_t, psj[0:64, :], rbc)
                        nc.scalar.dma_start(
                            attno_d[h4, 64 * hp:64 * hp + 64,
                                    SC * j:SC * (j + 1)], ao_t)

            # software-pipelined: proj/rope of group h4+1 is emitted before
            # attention of group h4 so its DVE/DMA work hides under PE time.
            # v heads 4..7 are deferred here (needed only from group 2 on) so
            # the first attention group starts ~14us earlier.
            qkT = emit_proj(0)
            nxt = emit_proj(1)
            with tc.tile_pool(name="wvb", bufs=1) as wvb_pool:
                wvb = wvb_pool.tile([P, NIC, HD // 2], F32R, tag="wvb")
                nc.sync.dma_start(
                    wvb, wv_d.rearrange("(ic p) o -> p ic o",
                                        p=P)[:, :, HD // 2:HD].bitcast(F32R))
                for st in range(NST):
                    psb = ps2_pool.tile([P, 2 * SC], F32, tag="ps2",
                                        name=f"psvb_{st}")
                    for ic in range(NIC):
                        nc.tensor.matmul(
                            psb[:, 0:HD // 2],
                            lhsT=_r(xT[:, ic, P * st:P * (st + 1)]),
                            rhs=_r(wvb[:, ic, :]),
                            start=(ic == 0), stop=(ic == NIC - 1))
                    nc.scalar.activation(
                        v_sb[:, st, HPC // 2:HPC, 0:64],
                        psb[:, 0:HD // 2].rearrange("p (h d) -> p h d",
                                                    h=HPC // 2),
                        func=AF.Copy)
            for h4 in range(NH4):
                if h4 == 0:
                    pass
                elif h4 + 1 < NH4:
                    nxt = emit_proj(h4 + 1)
                else:
                    nxt = None
                emit_attn(h4, qkT)
                qkT = nxt

    # ---------------- Phase 5: out-projection, yT = woT^T @ attnoT ------
    if "p5" in ABLATE:
        return
    with tc.tile_pool(name="wop", bufs=1) as wo_pool, \
         tc.tile_pool(name="aosc", bufs=2) as aosc_pool, \
         tc.tile_pool(name="ystage", bufs=3) as ystage_pool, \
         tc.tile_pool(name="psy", bufs=3, space="PSUM") as psy_pool:
        wo_sb = wo_pool.tile([P, NH4, D_MODEL], F32R, tag="wo")
        nc.sync.dma_start(wo_sb, wo_d.rearrange("(c p) o -> p c o", p=P).bitcast(F32R))
        for scp in range(NSC // 2):   # pairs of s-chunks, 1024-wide psum
            ao_sc = aosc_pool.tile([P, NH4, 2 * SC], F32R, tag="aosc")
            nc.sync.dma_start(
                ao_sc, attno_d.rearrange("c p s -> p c s")[:, :,
                                                           2 * SC * scp:2 * SC * (scp + 1)].bitcast(F32R))
            for ot in range(D_MODEL // P):
                psy = psy_pool.tile([P, 2 * SC], F32, tag="psy",
                                    name=f"psy_{scp}_{ot}")
                for half in range(2):
                    for c in range(NH4):
                        nc.tensor.matmul(
                            psy[:, SC * half:SC * (half + 1)],
                            lhsT=_r(wo_sb[:, c, P * ot:P * (ot + 1)]),
                            rhs=_r(ao_sc[:, c, SC * half:SC * (half + 1)]),
                            start=(c == 0), stop=(c == NH4 - 1))
                ystage = ystage_pool.tile([P, 2 * SC], F32, tag="ystage")
                nc.scalar.activation(ystage, psy, func=AF.Copy)
                nc.sync.dma_start(
                    y_d[P * ot:P * (ot + 1), 2 * SC * scp:2 * SC * (scp + 1)],
                    ystage)


# ---------------------------------------------------------------------------
# Host side
# ---------------------------------------------------------------------------

_NC_CACHE = {}


def _get_nc():
    if "nc" not in _NC_CACHE:
        _NC_CACHE["nc"] = build_nc()
    return _NC_CACHE["nc"]


def _perm64():
    # de-interleave: evens then odds, per 64-dim head
    return np.concatenate([np.arange(0, 64, 2), np.arange(1, 64, 2)])


def make_in_maps(x, token_positions, Wq, Wk, Wv, Wo):
    x = np.ascontiguousarray(np.asarray(x, dtype=np.float32))
    pos = np.asarray(token_positions).astype(np.float32)
    Wq = np.asarray(Wq, dtype=np.float32)
    Wk = np.asarray(Wk, dtype=np.float32)
    Wv = np.asarray(Wv, dtype=np.float32)
    Wo = np.asarray(Wo, dtype=np.float32)

    # RoPE tables in rotate-half (de-interleaved) form, [128, SEQ]:
    # rows 0:32 / 32:64 for head-low/high halves, repeated for partition 64:128
    inv_freq = (10000.0 ** (-np.arange(0, DK, 2, dtype=np.float32)
                            / np.float32(DK))).astype(np.float32)
    ang = inv_freq[:, None] * pos[None, :]            # [32, SEQ]
    cos = np.cos(ang).astype(np.float32)
    sin = np.sin(ang).astype(np.float32)
    cos_t = np.concatenate([cos, cos, cos, cos], axis=0)       # [128, SEQ]
    sin_t = np.concatenate([-sin, sin, -sin, sin], axis=0)     # [128, SEQ]

    # causal diagonal band mask: band[p, c] = 1 if p <= c  (one 128x128 tile)
    pidx = np.arange(P)[:, None]
    cidx = np.arange(P)[None, :]
    mask = (pidx <= cidx).astype(np.float32)

    ident = np.eye(P, dtype=np.float32)

    perm = _perm64()
    in_maps = []
    for c in range(N_CORES):
        b = c // 2
        hg = c % 2
        rows = slice(HD * hg, HD * (hg + 1))
        # per-head d-permutation for q/k
        qrows = (np.arange(HD).reshape(HPC, DK)[:, perm].reshape(HD)
                 + HD * hg)
        in_maps.append({
            "x": np.ascontiguousarray(x[b]),
            "wqT": np.ascontiguousarray(Wq[qrows, :].T),
            "wkT": np.ascontiguousarray(Wk[qrows, :].T),
            "wvT": np.ascontiguousarray(Wv[rows, :].T),
            "woT": np.ascontiguousarray(Wo[:, rows].T),
            "cosw": cos_t, "sinw": sin_t, "mask": mask, "ident": ident,
            "ones": np.ones((P, NST * HPC), dtype=np.float32),
        })
    return in_maps


def run(x, token_positions, Wq, Wk, Wv, Wo, trace=False):
    nc = _get_nc()
    in_maps = make_in_maps(x, token_positions, Wq, Wk, Wv, Wo)
    res = run_bass_kernel_spmd(nc, in_maps, list(range(N_CORES)),
                               trace=trace)
    parts = [r["yT"] for r in res.results]
    out = np.stack([(parts[2 * b] + parts[2 * b + 1]).T
                    for b in range(BATCH)]).astype(np.float32)
    return out, res


def kernel(x, token_positions, Wq, Wk, Wv, Wo):
    out, _ = run(x, token_positions, Wq, Wk, Wv, Wo, trace=False)
    return out



# revision 2
# speedup vs baseline: 1.2123x; 1.2123x over previous
"""Multi-head causal self-attention with RoPE on 8 Trainium2 NeuronCores.

Problem: x:(4,2048,1024) f32, 16 heads, d_k=64, causal, RoPE theta=1e4,
out = softmax(rope(q) rope(k)^T / 8, causal) v, then out-proj.

Sharding: core c handles batch c//2 and heads 8*(c%2) .. 8*(c%2)+8.
Each core computes QKV for its 8 heads (row-sliced weights), causal
attention, and a partial out-projection y_part = attnout_slice @ WoT_slice.
Host sums the two partials per batch.

v2 design (vs v1): all matmul operands bf16 (PSUM accum stays f32), x is
transposed on the host (no PE transposes), attention output stays in SBUF
(no DRAM staging round-trip), the softmax denominator comes from a
64-column ones block in the V stationary operand (rows 64:128 of the
attnV psum hold the denominator on 64 partitions -> plain DVE reciprocal,
no gpsimd partition broadcast), masks/zeros run on DVE in bf16, and the
out-projection is emitted per s-chunk between group-3 attention chunks
(j descending) so it fills PE while ACT drains the last exps.

Device layouts (per core):
  xT   [i, s]       - transposed activations (bf16, from host)
  qT,kT[hd, s]      - projections in transposed layout (RoPE'd in place)
  v_sb [s, st,h,128]- cols 0:64 v-dims, cols 64:128 ones (denominator)
  scoresT[sk, sq]   - psum; exp tiles feed attn@V directly as moving operand
  attno[hd, c, s]   - SBUF bf16, feeds out-proj; output written as yT[o, s]

The per-head d_k dims of Wq/Wk are host-permuted (evens then odds) so RoPE
becomes the rotate-half form; scores are invariant to this permutation.
"""

from contextlib import ExitStack

import ml_dtypes
import numpy as np

import concourse.tile as tile
from concourse import bacc, mybir
from concourse.bass_utils import run_bass_kernel_spmd

F32 = mybir.dt.float32
BF16 = mybir.dt.bfloat16
AF = mybir.ActivationFunctionType

D_MODEL = 1024
SEQ = 2048
BATCH = 4
N_HEADS = 16
DK = 64
N_CORES = 8
HPC = 8            # heads per core
HD = HPC * DK      # 512 head-dims per core
P = 128
SC = 512           # seq chunk (matmul moving dim)
NSC = SEQ // SC    # 4
NST = SEQ // P     # 16
NIC = D_MODEL // P # 8
NH4 = HD // P      # 4  (128-dim tiles = 2 heads each)


def build_nc():
    nc = bacc.Bacc("TRN2", target_bir_lowering=False, debug=False)

    xT_d = nc.dram_tensor("xT", [D_MODEL, SEQ], BF16, kind="ExternalInput").ap()
    wq_d = nc.dram_tensor("wqT", [D_MODEL, HD], BF16, kind="ExternalInput").ap()
    wk_d = nc.dram_tensor("wkT", [D_MODEL, HD], BF16, kind="ExternalInput").ap()
    wv_d = nc.dram_tensor("wvT", [D_MODEL, HD], BF16, kind="ExternalInput").ap()
    wo_d = nc.dram_tensor("woT", [HD, D_MODEL], BF16, kind="ExternalInput").ap()
    cos_d = nc.dram_tensor("cosw", [P, SEQ], BF16, kind="ExternalInput").ap()
    sin_d = nc.dram_tensor("sinw", [P, SEQ], BF16, kind="ExternalInput").ap()
    mask_d = nc.dram_tensor("mask", [P, P], BF16, kind="ExternalInput").ap()
    y_d = nc.dram_tensor("yT", [D_MODEL, SEQ], BF16, kind="ExternalOutput").ap()

    with tile.TileContext(nc) as tc:
        with ExitStack() as ctx:
            _emit(ctx, tc, xT_d, wq_d, wk_d, wv_d, wo_d, cos_d, sin_d,
                  mask_d, y_d)
    nc.compile()
    return nc


def _emit(ctx, tc, xT_d, wq_d, wk_d, wv_d, wo_d, cos_d, sin_d, mask_d, y_d):
    nc = tc.nc

    persist = ctx.enter_context(tc.tile_pool(name="persist", bufs=1))
    cos_sb = persist.tile([P, SEQ], BF16, tag="cos")
    sin_sb = persist.tile([P, SEQ], BF16, tag="sin")
    mask_sb = persist.tile([P, P], BF16, tag="mask")
    xT = persist.tile([P, NIC, SEQ], BF16, tag="xT")
    v_sb = persist.tile([P, NST, HPC, 2 * DK], BF16, tag="v")
    attno = persist.tile([P, NH4, SEQ], BF16, tag="attno")
    wo_sb = persist.tile([P, NH4, D_MODEL], BF16, tag="wo")
    wv_sb = persist.tile([P, NIC, HD], BF16, tag="wv")

    # DMA order on the sync queue is the arrival order.
    for c in range(2):
        nc.sync.dma_start(
            xT[:, :, SC * c:SC * (c + 1)],
            xT_d.rearrange("(ic p) s -> p ic s", p=P)[:, :, SC * c:SC * (c + 1)])
    nc.sync.dma_start(wv_sb, wv_d.rearrange("(ic p) o -> p ic o", p=P))
    for c in range(2, 4):
        nc.sync.dma_start(
            xT[:, :, SC * c:SC * (c + 1)],
            xT_d.rearrange("(ic p) s -> p ic s", p=P)[:, :, SC * c:SC * (c + 1)])
    nc.scalar.dma_start(cos_sb, cos_d)
    nc.scalar.dma_start(sin_sb, sin_d)
    nc.scalar.dma_start(mask_sb, mask_d)
    nc.scalar.dma_start(wo_sb, wo_d.rearrange("(c p) o -> p c o", p=P))

    # ones block for the softmax denominator (attnV psum rows 64:128)
    nc.vector.memset(v_sb[:, :, :, DK:2 * DK], 1.0)

    wqk_pool = ctx.enter_context(tc.tile_pool(name="wqk", bufs=3))
    qk_pool = ctx.enter_context(tc.tile_pool(name="qk", bufs=4))
    swp_pool = ctx.enter_context(tc.tile_pool(name="swp", bufs=2))
    exp_pool = ctx.enter_context(tc.tile_pool(name="exp", bufs=3))
    rec_pool = ctx.enter_context(tc.tile_pool(name="rec", bufs=3))
    ys_pool = ctx.enter_context(tc.tile_pool(name="ys", bufs=3))
    ps2_pool = ctx.enter_context(tc.tile_pool(name="ps2", bufs=2, space="PSUM"))
    psatt_pool = ctx.enter_context(tc.tile_pool(name="psatt", bufs=2,
                                                space="PSUM"))
    psy_pool = ctx.enter_context(tc.tile_pool(name="psy", bufs=2,
                                              space="PSUM"))

    def emit_vproj(st_lo, st_hi):
        for st in range(st_lo, st_hi):
            psv = psy_pool.tile([P, HD], F32, tag="psy", name=f"psv_{st}")
            for ic in range(NIC):
                nc.tensor.matmul(psv, lhsT=xT[:, ic, P * st:P * (st + 1)],
                                 rhs=wv_sb[:, ic, :],
                                 start=(ic == 0), stop=(ic == NIC - 1))
            nc.vector.tensor_copy(
                v_sb[:, st, :, 0:DK],
                psv[:].rearrange("p (h d) -> p h d", h=HPC))

    def emit_proj(h4, names=("q", "k"), qkT=None):
        # q/k projection for the 2-head group h4, RoPE fused:
        #   dstT = raw projection (psum evac, bf16); swp = partition-swapped
        #   raw; dstT = dstT*cos + swp*sin  (all DVE, bf16 4x mode)
        if qkT is None:
            qkT = {}
        for name in names:
            w_d = wq_d if name == "q" else wk_d
            w_t = wqk_pool.tile([P, NIC, P], BF16, tag="wqk")
            nc.sync.dma_start(
                w_t, w_d.rearrange("(ic p) o -> p ic o",
                                   p=P)[:, :, P * h4:P * (h4 + 1)])
            dstT = qk_pool.tile([P, SEQ], BF16, tag=f"{name}T",
                                name=f"{name}T_{h4}")
            qkT[name] = dstT
            swp = swp_pool.tile([P, SEQ], BF16, tag="swp",
                                name=f"swp_{h4}_{name}")
            for scp in range(2):   # pairs of s-chunks, 1024 wide
                ps2 = ps2_pool.tile([P, 2 * SC], F32, tag="ps2",
                                    name=f"ps2p_{h4}_{name}_{scp}")
                for half in range(2):
                    sc = 2 * scp + half
                    for ic in range(NIC):
                        nc.tensor.matmul(
                            ps2[:, SC * half:SC * (half + 1)],
                            lhsT=w_t[:, ic, :],
                            rhs=xT[:, ic, SC * sc:SC * (sc + 1)],
                            start=(ic == 0), stop=(ic == NIC - 1))
                chunk = slice(2 * SC * scp, 2 * SC * (scp + 1))
                nc.vector.tensor_copy(dstT[:, chunk], ps2)
                for (o, i) in ((0, 32), (32, 0), (64, 96), (96, 64)):
                    nc.sync.dma_start(swp[o:o + 32, chunk],
                                      dstT[i:i + 32, chunk])
            nc.vector.tensor_mul(dstT, dstT, cos_sb)
            nc.vector.tensor_mul(swp, swp, sin_sb)
            nc.vector.tensor_add(dstT, dstT, swp)
        return qkT

    def emit_attn_chunk(h4, qkT, j):
        # attention for the two heads of group h4, query chunk j.
        # sk-tiles t paired two per 2-bank psum: scoresT for (t, t+1) side
        # by side -> one exp -> two attnV accumulations into psj. Diagonal:
        #   pair (4j, 4j+1): full exp; zero cols [512,640); band masks at
        #     [0:128] (t=4j) and [640:768] (t=4j+1)
        #   pair (4j+2, 4j+3): halves restricted to >= 256; zero [768,896);
        #     bands at [256:384] and [896:1024]
        for hp in range(2):
            h = 2 * h4 + hp
            qh = qkT["q"][64 * hp:64 * hp + 64, :]
            kh = qkT["k"][64 * hp:64 * hp + 64, :]
            psj = psatt_pool.tile([P, SC], F32, tag="psatt",
                                  name=f"psatt_{h}_{j}")
            tmax = 4 * j + 3
            for tp in range(2 * j + 2):
                t0 = 2 * tp
                diag = t0 - 4 * j   # -4j..0..2: >=0 on diagonal
                kind = ("full" if diag < 0 else
                        "d01" if diag == 0 else "d23")
                n0 = 2 * P if kind == "d23" else 0
                ps2 = ps2_pool.tile([P, 2 * SC], F32, tag="ps2",
                                    name=f"ps2a_{h}_{j}_{tp}")
                for m in range(2):
                    t = t0 + m
                    nc.tensor.matmul(
                        ps2[:, SC * m + n0:SC * (m + 1)],
                        lhsT=kh[:, P * t:P * (t + 1)],
                        rhs=qh[:, SC * j + n0:SC * (j + 1)],
                        start=True, stop=True)
                exp2 = exp_pool.tile([P, 2 * SC], BF16, tag="exp",
                                     name=f"exp_{h}_{j}_{tp}")
                if kind == "d23":
                    # one ACT op over both 256-wide valid halves
                    nc.scalar.activation(
                        exp2[:].rearrange("p (b c) -> p b c", b=2)[:, :, n0:SC],
                        ps2[:].rearrange("p (b c) -> p b c", b=2)[:, :, n0:SC],
                        func=AF.Exp, scale=0.125)
                else:
                    nc.scalar.activation(exp2, ps2, func=AF.Exp, scale=0.125)
                if kind == "d01":
                    nc.vector.memset(exp2[:, SC:SC + P], 0.0)
                    nc.vector.tensor_mul(exp2[:, 0:P], exp2[:, 0:P], mask_sb)
                    nc.vector.tensor_mul(exp2[:, SC + P:SC + 2 * P],
                                         exp2[:, SC + P:SC + 2 * P], mask_sb)
                elif kind == "d23":
                    nc.vector.memset(exp2[:, SC + n0:SC + 3 * P], 0.0)
                    nc.vector.tensor_mul(exp2[:, n0:n0 + P],
                                         exp2[:, n0:n0 + P], mask_sb)
                    nc.vector.tensor_mul(exp2[:, SC + 3 * P:2 * SC],
                                         exp2[:, SC + 3 * P:2 * SC], mask_sb)
                for m in range(2):
                    t = t0 + m
                    out_ap = psj[:] if n0 == 0 else psj[:, n0:]
                    nc.tensor.matmul(
                        out_ap, lhsT=v_sb[:, t, h, :],
                        rhs=exp2[:, SC * m + n0:SC * (m + 1)],
                        start=(t == 0), stop=(t == tmax))
            # normalize + store this sq chunk into SBUF attno
            rec = rec_pool.tile([64, SC], F32, tag="rec", name=f"rec_{h}_{j}")
            nc.vector.reciprocal(rec, psj[64:128, :])
            nc.vector.tensor_mul(
                attno[64 * hp:64 * hp + 64, h4, SC * j:SC * (j + 1)],
                psj[0:64, :], rec)

    def emit_outproj(j):
        for ot in range(D_MODEL // P):
            psy = psy_pool.tile([P, SC], F32, tag="psy", name=f"psy_{j}_{ot}")
            for c in range(NH4):
                nc.tensor.matmul(
                    psy, lhsT=wo_sb[:, c, P * ot:P * (ot + 1)],
                    rhs=attno[:, c, SC * j:SC * (j + 1)],
                    start=(c == 0), stop=(c == NH4 - 1))
            ys = ys_pool.tile([P, SC], BF16, tag="ys", name=f"ys_{j}_{ot}")
            nc.vector.tensor_copy(ys, psy)
            nc.sync.dma_start(y_d[P * ot:P * (ot + 1), SC * j:SC * (j + 1)],
                              ys)

    # ---- emission schedule (per-engine FIFO order == execution order) ----
    qkT = emit_proj(0)
    emit_vproj(0, 8)
    nxt = emit_proj(1)

    # group 0: V remainder + nothing else to fill
    emit_attn_chunk(0, qkT, 0)
    emit_vproj(8, 12)
    emit_attn_chunk(0, qkT, 1)
    emit_vproj(12, 16)
    emit_attn_chunk(0, qkT, 2)
    emit_attn_chunk(0, qkT, 3)
    qkT = nxt

    # group 1 with group-2 proj split as filler
    emit_attn_chunk(1, qkT, 0)
    nxt = emit_proj(2, names=("q",))
    emit_attn_chunk(1, qkT, 1)
    emit_proj(2, names=("k",), qkT=nxt)
    emit_attn_chunk(1, qkT, 2)
    emit_attn_chunk(1, qkT, 3)
    qkT = nxt

    # group 2 with group-3 proj split as filler
    emit_attn_chunk(2, qkT, 0)
    nxt = emit_proj(3, names=("q",))
    emit_attn_chunk(2, qkT, 1)
    emit_proj(3, names=("k",), qkT=nxt)
    emit_attn_chunk(2, qkT, 2)
    emit_attn_chunk(2, qkT, 3)
    qkT = nxt

    # group 3 j descending, out-proj chunks interleaved as filler
    emit_attn_chunk(3, qkT, 3)
    emit_outproj(3)
    emit_attn_chunk(3, qkT, 2)
    emit_outproj(2)
    emit_attn_chunk(3, qkT, 1)
    emit_outproj(1)
    emit_attn_chunk(3, qkT, 0)
    emit_outproj(0)


# ---------------------------------------------------------------------------
# Host side
# ---------------------------------------------------------------------------

_NC_CACHE = {}


def _get_nc():
    if "nc" not in _NC_CACHE:
        _NC_CACHE["nc"] = build_nc()
    return _NC_CACHE["nc"]


def _perm64():
    # de-interleave: evens then odds, per 64-dim head
    return np.concatenate([np.arange(0, 64, 2), np.arange(1, 64, 2)])


def make_in_maps(x, token_positions, Wq, Wk, Wv, Wo):
    bf16 = ml_dtypes.bfloat16
    x = np.asarray(x, dtype=np.float32)
    pos = np.asarray(token_positions).astype(np.float32)
    Wq = np.asarray(Wq, dtype=np.float32)
    Wk = np.asarray(Wk, dtype=np.float32)
    Wv = np.asarray(Wv, dtype=np.float32)
    Wo = np.asarray(Wo, dtype=np.float32)

    # RoPE tables in rotate-half (de-interleaved) form, [128, SEQ]:
    # rows 0:32 / 32:64 for head-low/high halves, repeated for partition 64:128
    inv_freq = (10000.0 ** (-np.arange(0, DK, 2, dtype=np.float32)
                            / np.float32(DK))).astype(np.float32)
    ang = inv_freq[:, None] * pos[None, :]            # [32, SEQ]
    cos = np.cos(ang).astype(np.float32)
    sin = np.sin(ang).astype(np.float32)
    cos_t = np.concatenate([cos, cos, cos, cos], axis=0).astype(bf16)
    sin_t = np.concatenate([-sin, sin, -sin, sin], axis=0).astype(bf16)

    # causal diagonal band mask: band[p, c] = 1 if p <= c  (one 128x128 tile)
    pidx = np.arange(P)[:, None]
    cidx = np.arange(P)[None, :]
    mask = (pidx <= cidx).astype(bf16)

    perm = _perm64()
    in_maps = []
    for c in range(N_CORES):
        b = c // 2
        hg = c % 2
        rows = slice(HD * hg, HD * (hg + 1))
        # per-head d-permutation for q/k
        qrows = (np.arange(HD).reshape(HPC, DK)[:, perm].reshape(HD)
                 + HD * hg)
        in_maps.append({
            "xT": np.ascontiguousarray(x[b].T).astype(bf16),
            "wqT": np.ascontiguousarray(Wq[qrows, :].T).astype(bf16),
            "wkT": np.ascontiguousarray(Wk[qrows, :].T).astype(bf16),
            "wvT": np.ascontiguousarray(Wv[rows, :].T).astype(bf16),
            "woT": np.ascontiguousarray(Wo[:, rows].T).astype(bf16),
            "cosw": cos_t, "sinw": sin_t, "mask": mask,
        })
    return in_maps


def run(x, token_positions, Wq, Wk, Wv, Wo, trace=False):
    nc = _get_nc()
    in_maps = make_in_maps(x, token_positions, Wq, Wk, Wv, Wo)
    res = run_bass_kernel_spmd(nc, in_maps, list(range(N_CORES)),
                               trace=trace)
    parts = [np.asarray(r["yT"], dtype=np.float32) for r in res.results]
    out = np.stack([(parts[2 * b] + parts[2 * b + 1]).T
                    for b in range(BATCH)]).astype(np.float32)
    return out, res


def kernel(x, token_positions, Wq, Wk, Wv, Wo):
    out, _ = run(x, token_positions, Wq, Wk, Wv, Wo, trace=False)
    return out


# revision 7
# speedup vs baseline: 1.2214x; 1.0075x over previous
"""Multi-head causal self-attention with RoPE on 8 Trainium2 NeuronCores.

Problem: x:(4,2048,1024) f32, 16 heads, d_k=64, causal, RoPE theta=1e4,
out = softmax(rope(q) rope(k)^T / 8, causal) v, then out-proj.

Sharding: core c handles batch c//2 and heads 8*(c%2) .. 8*(c%2)+8.
Each core computes QKV for its 8 heads (row-sliced weights), causal
attention, and a partial out-projection y_part = attnout_slice @ WoT_slice.
Host sums the two partials per batch.

v2 design (vs v1): all matmul operands bf16 (PSUM accum stays f32), x is
transposed on the host (no PE transposes), attention output stays in SBUF
(no DRAM staging round-trip), the softmax denominator comes from a
64-column ones block in the V stationary operand (rows 64:128 of the
attnV psum hold the denominator on 64 partitions -> plain DVE reciprocal,
no gpsimd partition broadcast), masks/zeros run on DVE in bf16, and the
out-projection is emitted per s-chunk between group-3 attention chunks
(j descending) so it fills PE while ACT drains the last exps.

Device layouts (per core):
  xT   [i, s]       - transposed activations (bf16, from host)
  qT,kT[hd, s]      - projections in transposed layout (RoPE'd in place)
  v_sb [s, st,h,128]- cols 0:64 v-dims, cols 64:128 ones (denominator)
  scoresT[sk, sq]   - psum; exp tiles feed attn@V directly as moving operand
  attno[hd, c, s]   - SBUF bf16, feeds out-proj; output written as yT[o, s]

The per-head d_k dims of Wq/Wk are host-permuted (evens then odds) so RoPE
becomes the rotate-half form; scores are invariant to this permutation.
"""

from contextlib import ExitStack

import ml_dtypes
import numpy as np

import concourse.tile as tile
from concourse import bacc, mybir
from concourse.bass_utils import run_bass_kernel_spmd

F32 = mybir.dt.float32
BF16 = mybir.dt.bfloat16
AF = mybir.ActivationFunctionType

D_MODEL = 1024
SEQ = 2048
BATCH = 4
N_HEADS = 16
DK = 64
N_CORES = 8
HPC = 8            # heads per core
HD = HPC * DK      # 512 head-dims per core
P = 128
SC = 512           # seq chunk (matmul moving dim)
NSC = SEQ // SC    # 4
NST = SEQ // P     # 16
NIC = D_MODEL // P # 8
NH4 = HD // P      # 4  (128-dim tiles = 2 heads each)


def build_nc():
    nc = bacc.Bacc("TRN2", target_bir_lowering=False, debug=False)

    xT_d = nc.dram_tensor("xT", [D_MODEL, SEQ], BF16, kind="ExternalInput").ap()
    wq_d = nc.dram_tensor("wqT", [D_MODEL, HD], BF16, kind="ExternalInput").ap()
    wk_d = nc.dram_tensor("wkT", [D_MODEL, HD], BF16, kind="ExternalInput").ap()
    wv_d = nc.dram_tensor("wvT", [D_MODEL, HD], BF16, kind="ExternalInput").ap()
    wo_d = nc.dram_tensor("woT", [HD, D_MODEL], BF16, kind="ExternalInput").ap()
    cos_d = nc.dram_tensor("cosw", [P, SEQ], BF16, kind="ExternalInput").ap()
    sin_d = nc.dram_tensor("sinw", [P, SEQ], BF16, kind="ExternalInput").ap()
    mask_d = nc.dram_tensor("mask", [P, P], BF16, kind="ExternalInput").ap()
    y_d = nc.dram_tensor("yT", [D_MODEL, SEQ], BF16, kind="ExternalOutput").ap()

    with tile.TileContext(nc) as tc:
        with ExitStack() as ctx:
            _emit(ctx, tc, xT_d, wq_d, wk_d, wv_d, wo_d, cos_d, sin_d,
                  mask_d, y_d)
    nc.compile()
    return nc


def _emit(ctx, tc, xT_d, wq_d, wk_d, wv_d, wo_d, cos_d, sin_d, mask_d, y_d):
    nc = tc.nc

    persist = ctx.enter_context(tc.tile_pool(name="persist", bufs=1))
    cos_sb = persist.tile([P, SEQ], BF16, tag="cos")
    sin_sb = persist.tile([P, SEQ], BF16, tag="sin")
    mask_sb = persist.tile([P, P], BF16, tag="mask")
    xT = persist.tile([P, NIC, SEQ], BF16, tag="xT")
    v_sb = persist.tile([P, NST, HPC, 2 * DK], BF16, tag="v")
    attno = persist.tile([P, NH4, SEQ], BF16, tag="attno")
    wo_sb = persist.tile([P, NH4, D_MODEL], BF16, tag="wo")
    wv_sb = persist.tile([P, NIC, HD], BF16, tag="wv")

    wqk_pre = ctx.enter_context(tc.tile_pool(name="wqk_pre", bufs=1))
    w_pre = {}
    for name, w_d in (("q", wq_d), ("k", wk_d)):
        w_t = wqk_pre.tile([P, NIC, P], BF16, tag=f"w{name}0")
        nc.sync.dma_start(w_t,
                          w_d.rearrange("(ic p) o -> p ic o", p=P)[:, :, 0:P])
        w_pre[name] = w_t

    # DMA order on the sync queue is the arrival order: first q/k proj of
    # group 0 needs xT ic-halves of chunk 0 first, then the rest.
    xT_r = xT_d.rearrange("(ic p) s -> p ic s", p=P)
    nc.sync.dma_start(xT[:, 0:4, 0:SC], xT_r[:, 0:4, 0:SC])
    nc.sync.dma_start(xT[:, 4:8, 0:SC], xT_r[:, 4:8, 0:SC])
    nc.sync.dma_start(xT[:, :, SC:2 * SC], xT_r[:, :, SC:2 * SC])
    nc.sync.dma_start(wv_sb, wv_d.rearrange("(ic p) o -> p ic o", p=P))
    for c in range(2, 4):
        nc.sync.dma_start(xT[:, :, SC * c:SC * (c + 1)],
                          xT_r[:, :, SC * c:SC * (c + 1)])
    nc.scalar.dma_start(cos_sb, cos_d)
    nc.scalar.dma_start(sin_sb, sin_d)
    nc.scalar.dma_start(mask_sb, mask_d)
    nc.scalar.dma_start(wo_sb, wo_d.rearrange("(c p) o -> p c o", p=P))

    # ones block for the softmax denominator (attnV psum rows 64:128)
    nc.vector.memset(v_sb[:, :, :, DK:2 * DK], 1.0)

    wqk_pool = ctx.enter_context(tc.tile_pool(name="wqk", bufs=3))
    qk_pool = ctx.enter_context(tc.tile_pool(name="qk", bufs=4))
    swp_pool = ctx.enter_context(tc.tile_pool(name="swp", bufs=2))
    exp_pool = ctx.enter_context(tc.tile_pool(name="exp", bufs=3))
    rec_pool = ctx.enter_context(tc.tile_pool(name="rec", bufs=3))
    ys_pool = ctx.enter_context(tc.tile_pool(name="ys", bufs=3))
    ps2_pool = ctx.enter_context(tc.tile_pool(name="ps2", bufs=3, space="PSUM"))
    psatt_pool = ctx.enter_context(tc.tile_pool(name="psatt", bufs=2,
                                                space="PSUM"))

    def emit_vproj(st_lo, st_hi):
        for st in range(st_lo, st_hi):
            pst = ps2_pool.tile([P, 2 * SC], F32, tag="ps2", name=f"psv_{st}")
            psv = pst[:, 0:HD]
            for ic in range(NIC):
                nc.tensor.matmul(psv, lhsT=xT[:, ic, P * st:P * (st + 1)],
                                 rhs=wv_sb[:, ic, :],
                                 start=(ic == 0), stop=(ic == NIC - 1))
            nc.vector.tensor_copy(
                v_sb[:, st, :, 0:DK],
                psv.rearrange("p (h d) -> p h d", h=HPC))

    def emit_proj(h4, names=("q", "k"), qkT=None):
        # q/k projection for the 2-head group h4, RoPE fused:
        #   dstT = raw projection (psum evac, bf16); swp = partition-swapped
        #   raw; dstT = dstT*cos + swp*sin  (all DVE, bf16 4x mode)
        if qkT is None:
            qkT = {}
        for name in names:
            if h4 == 0:
                w_t = w_pre[name]
            else:
                w_d = wq_d if name == "q" else wk_d
                w_t = wqk_pool.tile([P, NIC, P], BF16, tag="wqk")
                nc.sync.dma_start(
                    w_t, w_d.rearrange("(ic p) o -> p ic o",
                                       p=P)[:, :, P * h4:P * (h4 + 1)])
            dstT = qk_pool.tile([P, SEQ], BF16, tag=f"{name}T",
                                name=f"{name}T_{h4}")
            qkT[name] = dstT
            swp = swp_pool.tile([P, SEQ], BF16, tag="swp",
                                name=f"swp_{h4}_{name}")
            for scp in range(2):   # pairs of s-chunks, 1024 wide
                ps2 = ps2_pool.tile([P, 2 * SC], F32, tag="ps2",
                                    name=f"ps2p_{h4}_{name}_{scp}")
                for half in range(2):
                    sc = 2 * scp + half
                    for ic in range(NIC):
                        nc.tensor.matmul(
                            ps2[:, SC * half:SC * (half + 1)],
                            lhsT=w_t[:, ic, :],
                            rhs=xT[:, ic, SC * sc:SC * (sc + 1)],
                            start=(ic == 0), stop=(ic == NIC - 1))
                chunk = slice(2 * SC * scp, 2 * SC * (scp + 1))
                nc.vector.tensor_copy(dstT[:, chunk], ps2)
                for (o, i) in ((0, 32), (32, 0), (64, 96), (96, 64)):
                    nc.sync.dma_start(swp[o:o + 32, chunk],
                                      dstT[i:i + 32, chunk])
            nc.vector.tensor_mul(dstT, dstT, cos_sb)
            nc.vector.tensor_mul(swp, swp, sin_sb)
            nc.vector.tensor_add(dstT, dstT, swp)
        return qkT

    def emit_attn_chunk(h4, qkT, j):
        # attention for the two heads of group h4, query chunk j.
        # sk-tiles t paired two per 2-bank psum: scoresT for (t, t+1) side
        # by side -> one exp -> two attnV accumulations into psj. Diagonal:
        #   pair (4j, 4j+1): full exp; zero cols [512,640); band masks at
        #     [0:128] (t=4j) and [640:768] (t=4j+1)
        #   pair (4j+2, 4j+3): halves restricted to >= 256; zero [768,896);
        #     bands at [256:384] and [896:1024]
        for hp in range(2):
            h = 2 * h4 + hp
            qh = qkT["q"][64 * hp:64 * hp + 64, :]
            kh = qkT["k"][64 * hp:64 * hp + 64, :]
            psj = psatt_pool.tile([P, SC], F32, tag="psatt",
                                  name=f"psatt_{h}_{j}")
            tmax = 4 * j + 3
            for tp in range(2 * j + 2):
                t0 = 2 * tp
                diag = t0 - 4 * j   # -4j..0..2: >=0 on diagonal
                kind = ("full" if diag < 0 else
                        "d01" if diag == 0 else "d23")
                n0 = 2 * P if kind == "d23" else 0
                ps2 = ps2_pool.tile([P, 2 * SC], F32, tag="ps2",
                                    name=f"ps2a_{h}_{j}_{tp}")
                for m in range(2):
                    t = t0 + m
                    nc.tensor.matmul(
                        ps2[:, SC * m + n0:SC * (m + 1)],
                        lhsT=kh[:, P * t:P * (t + 1)],
                        rhs=qh[:, SC * j + n0:SC * (j + 1)],
                        start=True, stop=True)
                exp2 = exp_pool.tile([P, 2 * SC], BF16, tag="exp",
                                     name=f"exp_{h}_{j}_{tp}")
                if kind == "d23":
                    # one ACT op over both 256-wide valid halves
                    nc.scalar.activation(
                        exp2[:].rearrange("p (b c) -> p b c", b=2)[:, :, n0:SC],
                        ps2[:].rearrange("p (b c) -> p b c", b=2)[:, :, n0:SC],
                        func=AF.Exp, scale=0.125)
                else:
                    nc.scalar.activation(exp2, ps2, func=AF.Exp, scale=0.125)
                if kind == "d01":
                    nc.vector.memset(exp2[:, SC:SC + P], 0.0)
                    nc.vector.tensor_mul(exp2[:, 0:P], exp2[:, 0:P], mask_sb)
                    nc.vector.tensor_mul(exp2[:, SC + P:SC + 2 * P],
                                         exp2[:, SC + P:SC + 2 * P], mask_sb)
                elif kind == "d23":
                    nc.vector.memset(exp2[:, SC + n0:SC + 3 * P], 0.0)
                    nc.vector.tensor_mul(exp2[:, n0:n0 + P],
                                         exp2[:, n0:n0 + P], mask_sb)
                    nc.vector.tensor_mul(exp2[:, SC + 3 * P:2 * SC],
                                         exp2[:, SC + 3 * P:2 * SC], mask_sb)
                for m in range(2):
                    t = t0 + m
                    out_ap = psj[:] if n0 == 0 else psj[:, n0:]
                    nc.tensor.matmul(
                        out_ap, lhsT=v_sb[:, t, h, :],
                        rhs=exp2[:, SC * m + n0:SC * (m + 1)],
                        start=(t == 0), stop=(t == tmax))
            # normalize + store this sq chunk into SBUF attno
            rec = rec_pool.tile([64, SC], F32, tag="rec", name=f"rec_{h}_{j}")
            nc.vector.reciprocal(rec, psj[64:128, :])
            nc.vector.tensor_mul(
                attno[64 * hp:64 * hp + 64, h4, SC * j:SC * (j + 1)],
                psj[0:64, :], rec)

    y_r = y_d.rearrange("(a p) s -> p a s", p=P)

    def emit_outproj(j):
        for otp in range(D_MODEL // (2 * P)):
            psy = ps2_pool.tile([P, 2 * SC], F32, tag="ps2",
                                name=f"psy_{j}_{otp}")
            for half in range(2):
                ot = 2 * otp + half
                for c in range(NH4):
                    nc.tensor.matmul(
                        psy[:, SC * half:SC * (half + 1)],
                        lhsT=wo_sb[:, c, P * ot:P * (ot + 1)],
                        rhs=attno[:, c, SC * j:SC * (j + 1)],
                        start=(c == 0), stop=(c == NH4 - 1))
            ys = ys_pool.tile([P, 2, SC], BF16, tag="ys", name=f"ys_{j}_{otp}")
            nc.vector.tensor_copy(ys, psy[:].rearrange("p (a s) -> p a s",
                                                       a=2))
            nc.sync.dma_start(
                y_r[:, 2 * otp:2 * otp + 2, SC * j:SC * (j + 1)], ys)

    # ---- emission schedule (per-engine FIFO order == execution order) ----
    qkT = emit_proj(0)
    emit_vproj(0, 8)
    nxt = emit_proj(1)

    # group 0: V remainder as filler
    emit_attn_chunk(0, qkT, 0)
    emit_vproj(8, 12)
    emit_attn_chunk(0, qkT, 1)
    emit_vproj(12, 16)
    emit_attn_chunk(0, qkT, 2)
    emit_attn_chunk(0, qkT, 3)
    qkT = nxt

    # group 1 with group-2 proj split as filler (k after j2: biggest lag)
    emit_attn_chunk(1, qkT, 0)
    nxt = emit_proj(2, names=("q",))
    emit_attn_chunk(1, qkT, 1)
    emit_attn_chunk(1, qkT, 2)
    emit_proj(2, names=("k",), qkT=nxt)
    emit_attn_chunk(1, qkT, 3)
    qkT = nxt

    # group 2 with group-3 proj split as filler
    emit_attn_chunk(2, qkT, 0)
    nxt = emit_proj(3, names=("q",))
    emit_attn_chunk(2, qkT, 1)
    emit_attn_chunk(2, qkT, 2)
    emit_proj(3, names=("k",), qkT=nxt)
    emit_attn_chunk(2, qkT, 3)
    qkT = nxt

    # group 3 j descending, out-proj chunks interleaved as filler
    emit_attn_chunk(3, qkT, 3)
    emit_outproj(3)
    emit_attn_chunk(3, qkT, 2)
    emit_outproj(2)
    emit_attn_chunk(3, qkT, 1)
    emit_outproj(1)
    emit_attn_chunk(3, qkT, 0)
    emit_outproj(0)


# ---------------------------------------------------------------------------
# Host side
# ---------------------------------------------------------------------------

_NC_CACHE = {}


def _get_nc():
    if "nc" not in _NC_CACHE:
        _NC_CACHE["nc"] = build_nc()
    return _NC_CACHE["nc"]


def _perm64():
    # de-interleave: evens then odds, per 64-dim head
    return np.concatenate([np.arange(0, 64, 2), np.arange(1, 64, 2)])


def make_in_maps(x, token_positions, Wq, Wk, Wv, Wo):
    bf16 = ml_dtypes.bfloat16
    x = np.asarray(x, dtype=np.float32)
    pos = np.asarray(token_positions).astype(np.float32)
    Wq = np.asarray(Wq, dtype=np.float32)
    Wk = np.asarray(Wk, dtype=np.float32)
    Wv = np.asarray(Wv, dtype=np.float32)
    Wo = np.asarray(Wo, dtype=np.float32)

    # RoPE tables in rotate-half (de-interleaved) form, [128, SEQ]:
    # rows 0:32 / 32:64 for head-low/high halves, repeated for partition 64:128
    inv_freq = (10000.0 ** (-np.arange(0, DK, 2, dtype=np.float32)
                            / np.float32(DK))).astype(np.float32)
    ang = inv_freq[:, None] * pos[None, :]            # [32, SEQ]
    cos = np.cos(ang).astype(np.float32)
    sin = np.sin(ang).astype(np.float32)
    cos_t = np.concatenate([cos, cos, cos, cos], axis=0).astype(bf16)
    sin_t = np.concatenate([-sin, sin, -sin, sin], axis=0).astype(bf16)

    # causal diagonal band mask: band[p, c] = 1 if p <= c  (one 128x128 tile)
    pidx = np.arange(P)[:, None]
    cidx = np.arange(P)[None, :]
    mask = (pidx <= cidx).astype(bf16)

    perm = _perm64()
    in_maps = []
    for c in range(N_CORES):
        b = c // 2
        hg = c % 2
        rows = slice(HD * hg, HD * (hg + 1))
        # per-head d-permutation for q/k
        qrows = (np.arange(HD).reshape(HPC, DK)[:, perm].reshape(HD)
                 + HD * hg)
        in_maps.append({
            "xT": np.ascontiguousarray(x[b].T).astype(bf16),
            "wqT": np.ascontiguousarray(Wq[qrows, :].T).astype(bf16),
            "wkT": np.ascontiguousarray(Wk[qrows, :].T).astype(bf16),
            "wvT": np.ascontiguousarray(Wv[rows, :].T).astype(bf16),
            "woT": np.ascontiguousarray(Wo[:, rows].T).astype(bf16),
            "cosw": cos_t, "sinw": sin_t, "mask": mask,
        })
    return in_maps


def run(x, token_positions, Wq, Wk, Wv, Wo, trace=False):
    nc = _get_nc()
    in_maps = make_in_maps(x, token_positions, Wq, Wk, Wv, Wo)
    res = run_bass_kernel_spmd(nc, in_maps, list(range(N_CORES)),
                               trace=trace)
    parts = [np.asarray(r["yT"], dtype=np.float32) for r in res.results]
    out = np.stack([(parts[2 * b] + parts[2 * b + 1]).T
                    for b in range(BATCH)]).astype(np.float32)
    return out, res


def kernel(x, token_positions, Wq, Wk, Wv, Wo):
    out, _ = run(x, token_positions, Wq, Wk, Wv, Wo, trace=False)
    return out


# revision 11
# speedup vs baseline: 1.3412x; 1.0981x over previous
"""Multi-head causal self-attention with RoPE on 8 Trainium2 NeuronCores.

Problem: x:(4,2048,1024) f32, 16 heads, d_k=64, causal, RoPE theta=1e4,
out = softmax(rope(q) rope(k)^T / 8, causal) v, then out-proj.

Sharding: core c handles batch c//2 and heads 8*(c%2) .. 8*(c%2)+8.
Each core computes QKV for its 8 heads (row-sliced weights), causal
attention, and a partial out-projection y_part = attnout_slice @ WoT_slice.
Host sums the two partials per batch.

v2 design (vs v1): all matmul operands bf16 (PSUM accum stays f32), x is
transposed on the host (no PE transposes), attention output stays in SBUF
(no DRAM staging round-trip), the softmax denominator comes from a
64-column ones block in the V stationary operand (rows 64:128 of the
attnV psum hold the denominator on 64 partitions -> plain DVE reciprocal,
no gpsimd partition broadcast), masks/zeros run on DVE in bf16, and the
out-projection is emitted per s-chunk between group-3 attention chunks
(j descending) so it fills PE while ACT drains the last exps.

Device layouts (per core):
  xT   [i, s]       - transposed activations (bf16, from host)
  qT,kT[hd, s]      - projections in transposed layout (RoPE'd in place)
  v_sb [s, st,h,128]- cols 0:64 v-dims, cols 64:128 ones (denominator)
  scoresT[sk, sq]   - psum; exp tiles feed attn@V directly as moving operand
  attno[hd, c, s]   - SBUF bf16, feeds out-proj; output written as yT[o, s]

The per-head d_k dims of Wq/Wk are host-permuted (evens then odds) so RoPE
becomes the rotate-half form; scores are invariant to this permutation.
"""

from contextlib import ExitStack

import ml_dtypes
import numpy as np

import concourse.tile as tile
from concourse import bacc, mybir
from concourse.bass_utils import run_bass_kernel_spmd

F32 = mybir.dt.float32
BF16 = mybir.dt.bfloat16
AF = mybir.ActivationFunctionType

D_MODEL = 1024
SEQ = 2048
BATCH = 4
N_HEADS = 16
DK = 64
N_CORES = 8
HPC = 8            # heads per core
HD = HPC * DK      # 512 head-dims per core
P = 128
SC = 512           # seq chunk (matmul moving dim)
NSC = SEQ // SC    # 4
NST = SEQ // P     # 16
NIC = D_MODEL // P # 8
NH4 = HD // P      # 4  (128-dim tiles = 2 heads each)


def build_nc():
    nc = bacc.Bacc("TRN2", target_bir_lowering=False, debug=False)

    xT_d = nc.dram_tensor("xT", [D_MODEL, SEQ], BF16, kind="ExternalInput").ap()
    wq_d = nc.dram_tensor("wqT", [D_MODEL, HD], BF16, kind="ExternalInput").ap()
    wk_d = nc.dram_tensor("wkT", [D_MODEL, HD], BF16, kind="ExternalInput").ap()
    wv_d = nc.dram_tensor("wvT", [D_MODEL, HD], BF16, kind="ExternalInput").ap()
    wo_d = nc.dram_tensor("woT", [HD, D_MODEL], BF16, kind="ExternalInput").ap()
    cos_d = nc.dram_tensor("cosw", [P, SEQ], BF16, kind="ExternalInput").ap()
    sin_d = nc.dram_tensor("sinw", [P, SEQ], BF16, kind="ExternalInput").ap()
    mask_d = nc.dram_tensor("mask", [P, P], BF16, kind="ExternalInput").ap()
    y_d = nc.dram_tensor("yT", [D_MODEL, SEQ], BF16, kind="ExternalOutput").ap()

    with tile.TileContext(nc) as tc:
        with ExitStack() as ctx:
            _emit(ctx, tc, xT_d, wq_d, wk_d, wv_d, wo_d, cos_d, sin_d,
                  mask_d, y_d)
    nc.compile()
    return nc


def _emit(ctx, tc, xT_d, wq_d, wk_d, wv_d, wo_d, cos_d, sin_d, mask_d, y_d):
    nc = tc.nc

    persist = ctx.enter_context(tc.tile_pool(name="persist", bufs=1))
    cos_sb = persist.tile([P, SEQ], BF16, tag="cos")
    sin_sb = persist.tile([P, SEQ], BF16, tag="sin")
    mask_sb = persist.tile([P, P], BF16, tag="mask")
    xT = persist.tile([P, NIC, SEQ], BF16, tag="xT")
    v_sb = persist.tile([P, NST, HPC, 2 * DK], BF16, tag="v")
    attno = persist.tile([P, NH4, SEQ], BF16, tag="attno")
    wo_sb = persist.tile([P, NH4, D_MODEL], BF16, tag="wo")
    wv_sb = persist.tile([P, NIC, HD], BF16, tag="wv")

    wqk_pre = ctx.enter_context(tc.tile_pool(name="wqk_pre", bufs=1))
    w_pre = {}
    for name, w_d in (("q", wq_d), ("k", wk_d)):
        w_t = wqk_pre.tile([P, NIC, P], BF16, tag=f"w{name}0")
        nc.sync.dma_start(w_t,
                          w_d.rearrange("(ic p) o -> p ic o", p=P)[:, :, 0:P])
        w_pre[name] = w_t

    # DMA order on the sync queue is the arrival order: first q/k proj of
    # group 0 needs xT ic-halves of chunk 0 first, then the rest.
    xT_r = xT_d.rearrange("(ic p) s -> p ic s", p=P)
    nc.sync.dma_start(xT[:, 0:4, 0:SC], xT_r[:, 0:4, 0:SC])
    nc.sync.dma_start(xT[:, 4:8, 0:SC], xT_r[:, 4:8, 0:SC])
    nc.sync.dma_start(xT[:, :, SC:2 * SC], xT_r[:, :, SC:2 * SC])
    nc.sync.dma_start(wv_sb, wv_d.rearrange("(ic p) o -> p ic o", p=P))
    for c in range(2, 4):
        nc.sync.dma_start(xT[:, :, SC * c:SC * (c + 1)],
                          xT_r[:, :, SC * c:SC * (c + 1)])
    nc.scalar.dma_start(cos_sb, cos_d)
    nc.scalar.dma_start(sin_sb, sin_d)
    nc.scalar.dma_start(mask_sb, mask_d)
    nc.scalar.dma_start(wo_sb, wo_d.rearrange("(c p) o -> p c o", p=P))

    # ones block for the softmax denominator (attnV psum rows 64:128)
    nc.vector.memset(v_sb[:, :, :, DK:2 * DK], 1.0)

    wqk_pool = ctx.enter_context(tc.tile_pool(name="wqk", bufs=3))
    qk_pool = ctx.enter_context(tc.tile_pool(name="qk", bufs=4))
    swp_pool = ctx.enter_context(tc.tile_pool(name="swp", bufs=2))
    exp_pool = ctx.enter_context(tc.tile_pool(name="exp", bufs=3))
    rec_pool = ctx.enter_context(tc.tile_pool(name="rec", bufs=3))
    ys_pool = ctx.enter_context(tc.tile_pool(name="ys", bufs=3))
    ps2_pool = ctx.enter_context(tc.tile_pool(name="ps2", bufs=3, space="PSUM"))
    psatt_pool = ctx.enter_context(tc.tile_pool(name="psatt", bufs=2,
                                                space="PSUM"))

    def emit_vproj(st_lo, st_hi):
        for st in range(st_lo, st_hi):
            pst = ps2_pool.tile([P, 2 * SC], F32, tag="ps2", name=f"psv_{st}")
            psv = pst[:, 0:HD]
            for ic in range(NIC):
                nc.tensor.matmul(psv, lhsT=xT[:, ic, P * st:P * (st + 1)],
                                 rhs=wv_sb[:, ic, :],
                                 start=(ic == 0), stop=(ic == NIC - 1))
            nc.vector.tensor_copy(
                v_sb[:, st, :, 0:DK],
                psv.rearrange("p (h d) -> p h d", h=HPC))

    def proj_prepare(h4, name):
        # issue the weight DMA and allocate tiles; MM/rope emission follows
        # later via proj_scp / proj_rope (possibly woven between attn pairs)
        if h4 == 0:
            w_t = w_pre[name]
        else:
            w_d = wq_d if name == "q" else wk_d
            w_t = wqk_pool.tile([P, NIC, P], BF16, tag="wqk")
            nc.sync.dma_start(
                w_t, w_d.rearrange("(ic p) o -> p ic o",
                                   p=P)[:, :, P * h4:P * (h4 + 1)])
        dstT = qk_pool.tile([P, SEQ], BF16, tag=f"{name}T",
                            name=f"{name}T_{h4}")
        swp = swp_pool.tile([P, SEQ], BF16, tag="swp",
                            name=f"swp_{h4}_{name}")
        return dict(h4=h4, name=name, w=w_t, d=dstT, s=swp)

    def proj_scp(st, scp):
        # one 1024-wide s-chunk pair of the projection + psum evac + swaps
        ps2 = ps2_pool.tile([P, 2 * SC], F32, tag="ps2",
                            name=f"ps2p_{st['h4']}_{st['name']}_{scp}")
        for half in range(2):
            sc = 2 * scp + half
            for ic in range(NIC):
                nc.tensor.matmul(
                    ps2[:, SC * half:SC * (half + 1)],
                    lhsT=st["w"][:, ic, :],
                    rhs=xT[:, ic, SC * sc:SC * (sc + 1)],
                    start=(ic == 0), stop=(ic == NIC - 1))
        chunk = slice(2 * SC * scp, 2 * SC * (scp + 1))
        nc.vector.tensor_copy(st["d"][:, chunk], ps2)
        for (o, i) in ((0, 32), (32, 0), (64, 96), (96, 64)):
            nc.sync.dma_start(st["s"][o:o + 32, chunk],
                              st["d"][i:i + 32, chunk])

    def proj_rope(st):
        # dstT = dstT*cos + swp*sin  (all DVE, bf16 4x mode)
        nc.vector.tensor_mul(st["d"], st["d"], cos_sb)
        nc.vector.tensor_mul(st["s"], st["s"], sin_sb)
        nc.vector.tensor_add(st["d"], st["d"], st["s"])

    def emit_proj(h4, names=("q", "k"), qkT=None):
        if qkT is None:
            qkT = {}
        for name in names:
            st = proj_prepare(h4, name)
            qkT[name] = st["d"]
            proj_scp(st, 0)
            proj_scp(st, 1)
            proj_rope(st)
        return qkT

    def emit_attn_chunk(h4, qkT, j, fillers=()):
        # attention for the two heads of group h4, query chunk j; the two
        # heads' chains are interleaved pair-by-pair so each engine always
        # has the other head's work while semaphores propagate. attnV lags
        # scores by one pair (software pipeline). `fillers` are PE filler
        # closures woven between pairs. Diagonal handling:
        #   pair (4j, 4j+1): full exp; zero cols [512,640); band masks at
        #     [0:128] (t=4j) and [640:768] (t=4j+1)
        #   pair (4j+2, 4j+3): halves restricted to >= 256; zero [768,896);
        #     bands at [256:384] and [896:1024]
        fillers = list(fillers)
        ntp = 2 * j + 2
        tmax = 4 * j + 3
        psj = [psatt_pool.tile([P, SC], F32, tag="psatt",
                               name=f"psatt_{2 * h4 + hp}_{j}")
               for hp in range(2)]

        def do_scores(hp, tp):
            h = 2 * h4 + hp
            qh = qkT["q"][64 * hp:64 * hp + 64, :]
            kh = qkT["k"][64 * hp:64 * hp + 64, :]
            t0 = 2 * tp
            diag = t0 - 4 * j   # -4j..0..2: >=0 on diagonal
            kind = ("full" if diag < 0 else "d01" if diag == 0 else "d23")
            n0 = 2 * P if kind == "d23" else 0
            ps2 = ps2_pool.tile([P, 2 * SC], F32, tag="ps2",
                                name=f"ps2a_{h}_{j}_{tp}")
            for m in range(2):
                t = t0 + m
                nc.tensor.matmul(
                    ps2[:, SC * m + n0:SC * (m + 1)],
                    lhsT=kh[:, P * t:P * (t + 1)],
                    rhs=qh[:, SC * j + n0:SC * (j + 1)],
                    start=True, stop=True)
            exp2 = exp_pool.tile([P, 2 * SC], BF16, tag="exp",
                                 name=f"exp_{h}_{j}_{tp}")
            if kind == "d23":
                # one ACT op over both 256-wide valid halves
                nc.scalar.activation(
                    exp2[:].rearrange("p (b c) -> p b c", b=2)[:, :, n0:SC],
                    ps2[:].rearrange("p (b c) -> p b c", b=2)[:, :, n0:SC],
                    func=AF.Exp, scale=0.125)
            else:
                nc.scalar.activation(exp2, ps2, func=AF.Exp, scale=0.125)
            if kind == "d01":
                nc.vector.memset(exp2[:, SC:SC + P], 0.0)
                nc.vector.tensor_mul(exp2[:, 0:P], exp2[:, 0:P], mask_sb)
                nc.vector.tensor_mul(exp2[:, SC + P:SC + 2 * P],
                                     exp2[:, SC + P:SC + 2 * P], mask_sb)
            elif kind == "d23":
                nc.vector.memset(exp2[:, SC + n0:SC + 3 * P], 0.0)
                nc.vector.tensor_mul(exp2[:, n0:n0 + P],
                                     exp2[:, n0:n0 + P], mask_sb)
                nc.vector.tensor_mul(exp2[:, SC + 3 * P:2 * SC],
                                     exp2[:, SC + 3 * P:2 * SC], mask_sb)
            return (hp, tp, exp2, n0)

        def do_attnv(hp, tp, exp2, n0):
            h = 2 * h4 + hp
            for m in range(2):
                t = 2 * tp + m
                out_ap = psj[hp][:] if n0 == 0 else psj[hp][:, n0:]
                nc.tensor.matmul(
                    out_ap, lhsT=v_sb[:, t, h, :],
                    rhs=exp2[:, SC * m + n0:SC * (m + 1)],
                    start=(t == 0), stop=(t == tmax))

        pend = []
        for tp in range(ntp):
            for hp in range(2):
                pend.append(do_scores(hp, tp))
            if tp >= 1:
                do_attnv(*pend.pop(0))
                do_attnv(*pend.pop(0))
            if fillers:
                fillers.pop(0)()
        while pend:
            do_attnv(*pend.pop(0))
        for f in fillers:
            f()
        # normalize + store this sq chunk into SBUF attno
        for hp in range(2):
            h = 2 * h4 + hp
            rec = rec_pool.tile([64, SC], F32, tag="rec", name=f"rec_{h}_{j}")
            nc.vector.reciprocal(rec, psj[hp][64:128, :])
            nc.vector.tensor_mul(
                attno[64 * hp:64 * hp + 64, h4, SC * j:SC * (j + 1)],
                psj[hp][0:64, :], rec)

    y_r = y_d.rearrange("(a p) s -> p a s", p=P)

    def outproj_unit(j, otp):
        psy = ps2_pool.tile([P, 2 * SC], F32, tag="ps2",
                            name=f"psy_{j}_{otp}")
        for half in range(2):
            ot = 2 * otp + half
            for c in range(NH4):
                nc.tensor.matmul(
                    psy[:, SC * half:SC * (half + 1)],
                    lhsT=wo_sb[:, c, P * ot:P * (ot + 1)],
                    rhs=attno[:, c, SC * j:SC * (j + 1)],
                    start=(c == 0), stop=(c == NH4 - 1))
        ys = ys_pool.tile([P, 2, SC], BF16, tag="ys", name=f"ys_{j}_{otp}")
        nc.vector.tensor_copy(ys, psy[:].rearrange("p (a s) -> p a s", a=2))
        nc.sync.dma_start(
            y_r[:, 2 * otp:2 * otp + 2, SC * j:SC * (j + 1)], ys)

    def emit_outproj(j):
        for otp in range(D_MODEL // (2 * P)):
            outproj_unit(j, otp)

    # ---- emission schedule (per-engine FIFO order == execution order) ----
    # group 0 proj interleaved with V proj so PE never waits on the x/wv
    # DMAs (q-scp0 only needs xT chunk 0; vproj st0-7 needs wv + chunks 0-1)
    st_q0 = proj_prepare(0, "q")
    st_k0 = proj_prepare(0, "k")
    qkT = {"q": st_q0["d"], "k": st_k0["d"]}
    proj_scp(st_q0, 0)
    proj_scp(st_k0, 0)
    emit_vproj(0, 4)
    proj_scp(st_q0, 1)
    proj_scp(st_k0, 1)
    proj_rope(st_q0)
    proj_rope(st_k0)
    emit_vproj(4, 8)
    nxt = emit_proj(1)

    # group 0: V remainder as filler
    emit_attn_chunk(0, qkT, 0)
    emit_vproj(8, 12)
    emit_attn_chunk(0, qkT, 1)
    emit_vproj(12, 16)
    emit_attn_chunk(0, qkT, 2)
    emit_attn_chunk(0, qkT, 3)
    qkT = nxt

    # groups 1, 2: next group's proj woven between attn pairs, one
    # 1024-wide scp unit per chunk (rope finishes inside j3)
    for g in (1, 2):
        st_q = proj_prepare(g + 1, "q")
        st_k = proj_prepare(g + 1, "k")
        nxt = {"q": st_q["d"], "k": st_k["d"]}
        emit_attn_chunk(g, qkT, 0, fillers=[lambda: proj_scp(st_q, 0)])
        emit_attn_chunk(g, qkT, 1, fillers=[lambda: proj_scp(st_q, 1)])
        emit_attn_chunk(g, qkT, 2, fillers=[lambda: (proj_scp(st_k, 0),
                                                     proj_rope(st_q))])
        emit_attn_chunk(g, qkT, 3, fillers=[lambda: (proj_scp(st_k, 1),
                                                     proj_rope(st_k))])
        qkT = nxt

    # group 3 j descending, out-proj chunks woven between attn pairs
    emit_attn_chunk(3, qkT, 3)
    emit_attn_chunk(3, qkT, 2,
                    fillers=[lambda o=o: outproj_unit(3, o) for o in range(4)])
    emit_attn_chunk(3, qkT, 1,
                    fillers=[lambda o=o: outproj_unit(2, o) for o in range(4)])
    emit_attn_chunk(3, qkT, 0,
                    fillers=[lambda o=o: outproj_unit(1, o) for o in range(4)])
    emit_outproj(0)


# ---------------------------------------------------------------------------
# Host side
# ---------------------------------------------------------------------------

_NC_CACHE = {}


def _get_nc():
    if "nc" not in _NC_CACHE:
        _NC_CACHE["nc"] = build_nc()
    return _NC_CACHE["nc"]


def _perm64():
    # de-interleave: evens then odds, per 64-dim head
    return np.concatenate([np.arange(0, 64, 2), np.arange(1, 64, 2)])


def make_in_maps(x, token_positions, Wq, Wk, Wv, Wo):
    bf16 = ml_dtypes.bfloat16
    x = np.asarray(x, dtype=np.float32)
    pos = np.asarray(token_positions).astype(np.float32)
    Wq = np.asarray(Wq, dtype=np.float32)
    Wk = np.asarray(Wk, dtype=np.float32)
    Wv = np.asarray(Wv, dtype=np.float32)
    Wo = np.asarray(Wo, dtype=np.float32)

    # RoPE tables in rotate-half (de-interleaved) form, [128, SEQ]:
    # rows 0:32 / 32:64 for head-low/high halves, repeated for partition 64:128
    inv_freq = (10000.0 ** (-np.arange(0, DK, 2, dtype=np.float32)
                            / np.float32(DK))).astype(np.float32)
    ang = inv_freq[:, None] * pos[None, :]            # [32, SEQ]
    cos = np.cos(ang).astype(np.float32)
    sin = np.sin(ang).astype(np.float32)
    cos_t = np.concatenate([cos, cos, cos, cos], axis=0).astype(bf16)
    sin_t = np.concatenate([-sin, sin, -sin, sin], axis=0).astype(bf16)

    # causal diagonal band mask: band[p, c] = 1 if p <= c  (one 128x128 tile)
    pidx = np.arange(P)[:, None]
    cidx = np.arange(P)[None, :]
    mask = (pidx <= cidx).astype(bf16)

    perm = _perm64()
    in_maps = []
    for c in range(N_CORES):
        b = c // 2
        hg = c % 2
        rows = slice(HD * hg, HD * (hg + 1))
        # per-head d-permutation for q/k
        qrows = (np.arange(HD).reshape(HPC, DK)[:, perm].reshape(HD)
                 + HD * hg)
        in_maps.append({
            "xT": np.ascontiguousarray(x[b].T).astype(bf16),
            "wqT": np.ascontiguousarray(Wq[qrows, :].T).astype(bf16),
            "wkT": np.ascontiguousarray(Wk[qrows, :].T).astype(bf16),
            "wvT": np.ascontiguousarray(Wv[rows, :].T).astype(bf16),
            "woT": np.ascontiguousarray(Wo[:, rows].T).astype(bf16),
            "cosw": cos_t, "sinw": sin_t, "mask": mask,
        })
    return in_maps


def run(x, token_positions, Wq, Wk, Wv, Wo, trace=False):
    nc = _get_nc()
    in_maps = make_in_maps(x, token_positions, Wq, Wk, Wv, Wo)
    res = run_bass_kernel_spmd(nc, in_maps, list(range(N_CORES)),
                               trace=trace)
    parts = [np.asarray(r["yT"], dtype=np.float32) for r in res.results]
    out = np.stack([(parts[2 * b] + parts[2 * b + 1]).T
                    for b in range(BATCH)]).astype(np.float32)
    return out, res


def kernel(x, token_positions, Wq, Wk, Wv, Wo):
    out, _ = run(x, token_positions, Wq, Wk, Wv, Wo, trace=False)
    return out
